# revision 13
# baseline (speedup 1.0000x reference)
"""Trainium2 Bass kernel for nn_Attention_80092550136278.

Gated attention with pair bias:
  q = (q_data @ Wq) * d^-0.5 ; k = k_data @ Wk ; v = v_data @ Wv   (per head)
  w = softmax(q k^T + pair_bias) ; ctx = w @ v
  out = (ctx * sigmoid(q_data @ Wg.T + gating_b)) @ Wo.T + o_bias

Sharding: sequence-parallel over q across 8 NeuronCores (256 q rows/core).
Each core gets the full K/V (replicated) and its slice of q/pair_bias.
No collectives; outputs are concatenated on host.

v2 design (vs the transpose-based v1 at 172us):
  * logits are computed TRANSPOSED directly on the PE:
      sT[k_chunk, q] = k_projT[d, k_chunk]^T @ q_projT[d, q]
    eliminating all 256 PE transposes (which cost ~70us incl. LDWEIGHTS).
  * pair_bias is pre-processed on the host into exp(bias)^T, tiled to the
    exact SBUF layout, cast to bf16 (halves the dominant DMA stream):
      softmax numerator = exp(s + b) = exp(s) * exp(b)
    so ACT does exp(psum logits)->sbuf bf16 (also serving as the PSUM
    eviction) and DVE does a bf16 2x-rate multiply with the staged expb.
  * everything on the matmul path is bf16 (FWL fast weight loads; fp32r
    would stream the same 1 col/cycle but pays full-rate LDWEIGHTS).
  * sigmoid(g) = 0.5 + 0.5*tanh(0.5*g): tanh lives in the same ACT table
    set as exp -> single ACT_TABLE_LOAD. gating_b is folded in via a
    rank-1 ones-outer-product matmul into the gate PSUM accumulation.
  * ctx matmul keeps the ones-column-in-v trick: row 32 of ctx^T PSUM is
    the softmax denominator for free.
  * dense PE issue order keeps the HAM clock gate at 2.4 GHz (v1 spent
    109us of 172us throttled to 1.2 GHz).
"""

import numpy as np

H, D, NQT, NK, C = 8, 32, 2048, 2048, 256
NQ = NQT // 8          # q rows per core
SCALE = D ** -0.5

_CACHE = {}


def _build_nc():
    import concourse.bass as bass
    import concourse.bacc as bacc
    import concourse.tile as tile
    import concourse.mybir as mybir

    F32 = mybir.dt.float32
    BF16 = mybir.dt.bfloat16
    AF = mybir.ActivationFunctionType
    ALU = mybir.AluOpType

    nc = bacc.Bacc("TRN2", debug=False)

    # ---- DRAM I/O ----
    d_qT = nc.dram_tensor("qT", [C, NQ], BF16, kind="ExternalInput")
    d_kT = nc.dram_tensor("kT", [C, NK], BF16, kind="ExternalInput")
    d_vT = nc.dram_tensor("vT", [C, NK], BF16, kind="ExternalInput")
    d_expb = nc.dram_tensor("expb", [H, 128, 16 * NQ], BF16, kind="ExternalInput")
    d_wq = nc.dram_tensor("wq", [C, C], BF16, kind="ExternalInput")
    d_wk = nc.dram_tensor("wk", [C, C], BF16, kind="ExternalInput")
    d_wv = nc.dram_tensor("wv", [C, C], BF16, kind="ExternalInput")
    d_wgT = nc.dram_tensor("wgT", [C, C], BF16, kind="ExternalInput")
    d_woT = nc.dram_tensor("woT", [H, D, C], BF16, kind="ExternalInput")
    d_gb = nc.dram_tensor("gb", [1, H * D], BF16, kind="ExternalInput")
    d_ob = nc.dram_tensor("ob", [1, C], BF16, kind="ExternalInput")
    d_ones = nc.dram_tensor("ones", [128, 256], BF16, kind="ExternalInput")
    d_out = nc.dram_tensor("out", [NQ, C], F32, kind="ExternalOutput")

    with tile.TileContext(nc) as tc:
        with tc.tile_pool(name="persist", bufs=1) as pers:

            # ---------------- persistent SBUF ----------------
            q_projT = [pers.tile([128, NQ], BF16, name=f"q_projT{i}") for i in range(2)]
            k_projT = [pers.tile([128, NK], BF16, name=f"k_projT{i}") for i in range(2)]
            v_aug = pers.tile([128, 16 * H * 33], BF16, name="v_aug")
            gate_t = pers.tile([D, H * NQ], F32, name="gate_t")
            gate_sb = pers.tile([D, H * NQ], F32, name="gate_sb")
            comb = pers.tile([D, H * NQ], BF16, name="comb")
            woT_sb = [pers.tile([D, C], BF16, name=f"woT{h}") for h in range(H)]
            gb_sb = pers.tile([1, H * D], BF16, name="gb_sb")
            ob_sb = pers.tile([1, C], BF16, name="ob_sb")
            ones_sb = pers.tile([128, 256], BF16, name="ones_sb")
            rs16 = pers.tile([33, H * NQ], BF16, name="rs16")
            rsr = pers.tile([D, H * NQ], F32, name="rsr")
            cg = pers.tile([D, NQ], F32, name="cg")
            out_sb = [pers.tile([128, C], F32, name=f"out_sb{i}") for i in range(2)]

            # expb staging: all 8 heads resident (64 KB/partition). Head 0 goes
            # on the sync (HWDGE) ring BEHIND the stage-1 inputs so projections
            # are never starved; heads 1-7 stream on the gpsimd (SWDGE) ring
            # from t=0 and finish well before they are consumed.
            bias_pool = tc.alloc_tile_pool(name="bias_sb", bufs=8)
            bias_tiles = {}

            with tc.tile_pool(name="stage1_sb", bufs=1) as s1, \
                 tc.tile_pool(name="s1_big_ps", bufs=2, space="PSUM") as bigp, \
                 tc.tile_pool(name="s1_small_ps", bufs=2, space="PSUM") as smallp, \
                 tc.tile_pool(name="s1_gate_ps", bufs=2, space="PSUM") as gatep:
                qT_sb = [s1.tile([128, NQ], BF16, name=f"qT{i}") for i in range(2)]
                kT_sb = [s1.tile([128, NK], BF16, name=f"kT{i}") for i in range(2)]
                vT_sb = [s1.tile([128, NK], BF16, name=f"vT{i}") for i in range(2)]
                wq_sb = [s1.tile([128, C], BF16, name=f"wq{i}") for i in range(2)]
                wk_sb = [s1.tile([128, C], BF16, name=f"wk{i}") for i in range(2)]
                wv_sb = [s1.tile([128, C], BF16, name=f"wv{i}") for i in range(2)]
                wgT_sb = [s1.tile([128, C], BF16, name=f"wgT{i}") for i in range(2)]
                for i in range(2):
                    sl = slice(i * 128, (i + 1) * 128)
                    nc.sync.dma_start(qT_sb[i][:], d_qT.ap()[sl, :])
                    nc.sync.dma_start(wq_sb[i][:], d_wq.ap()[sl, :])
                    nc.sync.dma_start(kT_sb[i][:], d_kT.ap()[sl, :])
                    nc.sync.dma_start(wk_sb[i][:], d_wk.ap()[sl, :])
                    nc.sync.dma_start(vT_sb[i][:], d_vT.ap()[sl, :])
                    nc.sync.dma_start(wv_sb[i][:], d_wv.ap()[sl, :])
                    nc.sync.dma_start(wgT_sb[i][:], d_wgT.ap()[sl, :])
                for h in range(H):
                    nc.sync.dma_start(woT_sb[h][:], d_woT.ap()[h])
                nc.sync.dma_start(gb_sb[:], d_gb.ap()[:])
                nc.sync.dma_start(ob_sb[:], d_ob.ap()[:])
                nc.sync.dma_start(ones_sb[:], d_ones.ap()[:])
                for h in range(H):
                    t = bias_pool.tile([128, 16 * NQ], BF16, tag="bias", name="bias_t")
                    eng = nc.sync if h == 0 else nc.gpsimd
                    eng.dma_start(t[:], d_expb.ap()[h])
                    bias_tiles[h] = t

                # ones columns of v_aug (position 32 of each 33-wide head block)
                v_aug4 = v_aug.rearrange("p (n h e) -> p n h e", n=16, h=H)
                nc.vector.tensor_copy(
                    v_aug4[:, :, :, D:D + 1],
                    ones_sb[:, 0:16 * H].rearrange("p (n h) -> p n h", n=16).unsqueeze(-1))

                # q_projT[mh] [128, NQ] = (Wq_s.T @ q_data.T) chunk (4 heads per tile)
                for mh in range(2):
                    pq = smallp.tile([128, NQ], F32, tag="proj", name="pq")
                    for kc in range(2):
                        nc.tensor.matmul(pq[:], wq_sb[kc][:, mh * 128:(mh + 1) * 128],
                                         qT_sb[kc][:], start=(kc == 0), stop=(kc == 1))
                    nc.vector.tensor_copy(q_projT[mh][:], pq[:])

                # k_projT[mh] [128, NK], evicted by ACT (idle in stage 1)
                for mh in range(2):
                    for half in range(2):
                        pk = bigp.tile([128, 1024], F32, tag="big", name="pk")
                        for nn in range(2):
                            for kc in range(2):
                                nc.tensor.matmul(
                                    pk[:, nn * 512:(nn + 1) * 512],
                                    wk_sb[kc][:, mh * 128:(mh + 1) * 128],
                                    kT_sb[kc][:, half * 1024 + nn * 512:half * 1024 + (nn + 1) * 512],
                                    start=(kc == 0), stop=(kc == 1))
                        nc.scalar.copy(k_projT[mh][:, half * 1024:(half + 1) * 1024], pk[:])

                # v_proj natural layout -> scatter into v_aug (ACT strided copy)
                for g in range(4):
                    pv = bigp.tile([128, 1024], F32, tag="big", name="pv")
                    for nn in range(4):
                        for kc in range(2):
                            nc.tensor.matmul(
                                pv[:, nn * 256:(nn + 1) * 256],
                                vT_sb[kc][:, (g * 4 + nn) * 128:(g * 4 + nn + 1) * 128],
                                wv_sb[kc][:], start=(kc == 0), stop=(kc == 1))
                    nc.scalar.copy(
                        v_aug4[:, g * 4:(g + 1) * 4, :, 0:D],
                        pv[:].rearrange("p (n h d) -> p n h d", n=4, h=H))

                # pin the ACT table set to exp_and_others (holds exp AND tanh)
                # before the first tanh, via a tiny dummy exp
                nc.scalar.activation(gate_t[0:1, 0:1], ones_sb[0:1, 0:1], AF.Exp)

                # gate: per head psum [32, NQ] = WgT_h.T @ qT + gb_h x ones,
                # then tanh(0.5*x) on ACT; sigmoid = 0.5 + 0.5*tanh below
                for h in range(H):
                    pg = gatep.tile([D, NQ], F32, tag="gate", name="pg")
                    for kc in range(2):
                        nc.tensor.matmul(pg[:], wgT_sb[kc][:, h * D:(h + 1) * D],
                                         qT_sb[kc][:], start=(kc == 0), stop=False)
                    nc.tensor.matmul(pg[:], gb_sb[:, h * D:(h + 1) * D],
                                     ones_sb[0:1, 0:NQ], start=False, stop=True)
                    nc.scalar.activation(gate_t[:, h * NQ:(h + 1) * NQ],
                                         pg[:], AF.Tanh, scale=0.5)
                nc.vector.tensor_scalar(gate_sb[:], gate_t[:], 0.5, 0.5,
                                        ALU.mult, ALU.add)

            # ---------------- stage 2+3: attention ----------------
            wTe_pool = tc.alloc_tile_pool(name="wTe_sb", bufs=2)
            wT_pool = tc.alloc_tile_pool(name="wT_sb", bufs=2)
            pl_pool = tc.alloc_tile_pool(name="pl", bufs=3, space="PSUM")
            pc_pool = tc.alloc_tile_pool(name="pc", bufs=2, space="PSUM")

            for th in range(2):
                for i in range(4):
                    h = 4 * th + i
                    po = i * 32
                    wTe = wTe_pool.tile([128, 16 * NQ], BF16, tag="wTe", name="wTe")
                    wT = wT_pool.tile([128, 16 * NQ], BF16, tag="wT", name="wT")
                    expb_t = bias_tiles[h]
                    pctx = pc_pool.tile([128, NQ], F32, tag="pc", name="pctx")
                    for qt in range(4):
                        pl = pl_pool.tile([128, 1024], F32, tag="pl", name="pl")
                        for j in range(4):
                            kc = qt * 4 + j
                            nc.tensor.matmul(
                                pl[:, j * NQ:(j + 1) * NQ],
                                k_projT[th][po:po + 32, kc * 128:(kc + 1) * 128],
                                q_projT[th][po:po + 32, :],
                                start=True, stop=True, tile_position=(po, 0))
                        qsl = slice(qt * 1024, (qt + 1) * 1024)
                        nc.scalar.activation(wTe[:, qsl], pl[:], AF.Exp)
                        nc.vector.tensor_mul(wT[:, qsl], wTe[:, qsl], expb_t[:, qsl])
                        for j in range(4):
                            kc = qt * 4 + j
                            nc.tensor.matmul(
                                pctx[0:33, :],
                                v_aug[:, kc * (H * 33) + h * 33: kc * (H * 33) + h * 33 + 33],
                                wT[:, kc * NQ:(kc + 1) * NQ],
                                start=(kc == 0), stop=(kc == 15))
                    # denominators: raw sums sit in row 32 of pctx. Broadcast to
                    # 32 partitions via ones outer product, then fast reciprocal
                    # at partition offset 0.
                    nc.vector.tensor_copy(rs16[32:33, h * NQ:(h + 1) * NQ], pctx[32:33, :])
                    prsb = pc_pool.tile([128, NQ], F32, tag="pc", name="prsb")
                    nc.tensor.matmul(prsb[0:32, :], ones_sb[32:33, 0:32],
                                     rs16[32:33, h * NQ:(h + 1) * NQ],
                                     start=True, stop=True, tile_position=(32, 0))
                    nc.vector.reciprocal_approx_fast(
                        out=rsr[:, h * NQ:(h + 1) * NQ], in_=prsb[0:32, :])
                    # comb = ctx * gate * recip
                    nc.vector.tensor_mul(cg[:], pctx[0:32, :], gate_sb[:, h * NQ:(h + 1) * NQ])
                    nc.vector.tensor_mul(comb[:, h * NQ:(h + 1) * NQ], cg[:],
                                         rsr[:, h * NQ:(h + 1) * NQ])

            # ---------------- stage 4: output projection ----------------
            for qm in range(2):
                pout = pc_pool.tile([128, C], F32, tag="pc", name="pout")
                for h in range(H):
                    nc.tensor.matmul(pout[:],
                                     comb[:, h * NQ + qm * 128: h * NQ + qm * 128 + 128],
                                     woT_sb[h][:], start=(h == 0), stop=False)
                # o_bias outer product: ones[1,128].T @ ob[1,256]
                nc.tensor.matmul(pout[:], ones_sb[0:1, 0:128], ob_sb[:],
                                 start=False, stop=True)
                nc.vector.tensor_copy(out_sb[qm][:], pout[:])
                nc.sync.dma_start(d_out.ap()[qm * 128:(qm + 1) * 128, :], out_sb[qm][:])

            pc_pool.release()
            pl_pool.release()
            wT_pool.release()
            wTe_pool.release()
            bias_pool.release()

    nc.compile()
    return nc


def _prep_in_maps(inputs):
    import ml_dtypes
    BF = ml_dtypes.bfloat16

    q_data = np.asarray(inputs["q_data"], dtype=np.float32)
    k_data = np.asarray(inputs["k_data"], dtype=np.float32)
    v_data = np.asarray(inputs["v_data"], dtype=np.float32)
    pair_bias = np.asarray(inputs["pair_bias"], dtype=np.float32)
    Wq = np.asarray(inputs["Wq"], dtype=np.float32)
    Wk = np.asarray(inputs["Wk"], dtype=np.float32)
    Wv = np.asarray(inputs["Wv"], dtype=np.float32)
    Wg = np.asarray(inputs["Wg"], dtype=np.float32)
    Wo = np.asarray(inputs["Wo"], dtype=np.float32)
    gating_b = np.asarray(inputs["gating_b"], dtype=np.float32)
    o_bias = np.asarray(inputs["o_bias"], dtype=np.float32)

    kT = np.ascontiguousarray(k_data.T).astype(BF)
    vT = np.ascontiguousarray(v_data.T).astype(BF)
    wq = np.ascontiguousarray(Wq * np.float32(SCALE)).astype(BF)
    wk = Wk.astype(BF)
    wv = Wv.astype(BF)
    wgT = np.ascontiguousarray(Wg.T).astype(BF)
    woT = np.ascontiguousarray(Wo.T.reshape(H, D, C)).astype(BF)
    gb = gating_b.reshape(1, H * D).astype(BF)
    ob = o_bias.reshape(1, C).astype(BF)
    ones = np.ones((128, 256), dtype=BF)

    # exp(pair_bias) transposed and tiled to the exact SBUF layout:
    # expb[c][h, p, kc*NQ + q] = exp(pair_bias[h, c*NQ + q, kc*128 + p])
    expb_all = np.exp(pair_bias)  # [H, NQT, NK]

    in_maps = []
    for c in range(8):
        qs = slice(c * NQ, (c + 1) * NQ)
        eb = expb_all[:, qs, :]                      # [H, NQ, NK]
        eb = eb.transpose(0, 2, 1)                   # [H, NK, NQ]
        eb = eb.reshape(H, 16, 128, NQ).transpose(0, 2, 1, 3)  # [H, 128, 16, NQ]
        eb = np.ascontiguousarray(eb.reshape(H, 128, 16 * NQ)).astype(BF)
        in_maps.append(dict(
            qT=np.ascontiguousarray(q_data[qs, :].T).astype(BF),
            kT=kT, vT=vT,
            expb=eb,
            wq=wq, wk=wk, wv=wv, wgT=wgT, woT=woT,
            gb=gb, ob=ob, ones=ones,
        ))
    return in_maps


def _get_nc():
    if "nc" not in _CACHE:
        _CACHE["nc"] = _build_nc()
    return _CACHE["nc"]


def _run(inputs, trace=False, trace_cores=None):
    from concourse import bass_utils
    nc = _get_nc()
    in_maps = _prep_in_maps(inputs)
    kwargs = {}
    if trace:
        kwargs = dict(trace=True, trace_cores=trace_cores or [0])
    res = bass_utils.run_bass_kernel_spmd(nc, in_maps, core_ids=list(range(8)), **kwargs)
    out = np.concatenate([res.results[c]["out"] for c in range(8)], axis=0)
    return out, res


def kernel(**inputs) -> np.ndarray:
    out, _ = _run(inputs)
    return out


# revision 15
# speedup vs baseline: 1.3567x; 1.3567x over previous
"""Trainium2 Bass kernel for nn_Attention_80092550136278.

Gated attention with pair bias:
  q = (q_data @ Wq) * d^-0.5 ; k = k_data @ Wk ; v = v_data @ Wv   (per head)
  w = softmax(q k^T + pair_bias) ; ctx = w @ v
  out = (ctx * sigmoid(q_data @ Wg.T + gating_b)) @ Wo.T + o_bias

Sharding: sequence-parallel over q across 8 NeuronCores (256 q rows/core).
Each core gets the full K/V (replicated) and its slice of q/pair_bias.
No collectives; outputs are concatenated on host.

v2 design (vs the transpose-based v1 at 172us):
  * logits are computed TRANSPOSED directly on the PE:
      sT[k_chunk, q] = k_projT[d, k_chunk]^T @ q_projT[d, q]
    eliminating all 256 PE transposes (which cost ~70us incl. LDWEIGHTS).
  * pair_bias is pre-processed on the host into exp(bias)^T, tiled to the
    exact SBUF layout, cast to bf16 (halves the dominant DMA stream):
      softmax numerator = exp(s + b) = exp(s) * exp(b)
    so ACT does exp(psum logits)->sbuf bf16 (also serving as the PSUM
    eviction) and DVE does a bf16 2x-rate multiply with the staged expb.
  * everything on the matmul path is bf16 (FWL fast weight loads; fp32r
    would stream the same 1 col/cycle but pays full-rate LDWEIGHTS).
  * sigmoid(g) = 0.5 + 0.5*tanh(0.5*g): tanh lives in the same ACT table
    set as exp -> single ACT_TABLE_LOAD. gating_b is folded in via a
    rank-1 ones-outer-product matmul into the gate PSUM accumulation.
  * ctx matmul keeps the ones-column-in-v trick: row 32 of ctx^T PSUM is
    the softmax denominator for free.
  * dense PE issue order keeps the HAM clock gate at 2.4 GHz (v1 spent
    109us of 172us throttled to 1.2 GHz).
"""

import numpy as np

H, D, NQT, NK, C = 8, 32, 2048, 2048, 256
NQ = NQT // 8          # q rows per core
SCALE = D ** -0.5

_CACHE = {}


def _build_nc():
    import concourse.bass as bass
    import concourse.bacc as bacc
    import concourse.tile as tile
    import concourse.mybir as mybir

    F32 = mybir.dt.float32
    BF16 = mybir.dt.bfloat16
    AF = mybir.ActivationFunctionType
    ALU = mybir.AluOpType

    nc = bacc.Bacc("TRN2", debug=False)

    # ---- DRAM I/O ----
    d_qT = nc.dram_tensor("qT", [C, NQ], BF16, kind="ExternalInput")
    d_kT = nc.dram_tensor("kT", [C, NK], BF16, kind="ExternalInput")
    d_vT = nc.dram_tensor("vT", [C, NK], BF16, kind="ExternalInput")
    d_expb = nc.dram_tensor("expb", [H, 128, 16 * NQ], BF16, kind="ExternalInput")
    d_wq = nc.dram_tensor("wq", [C, C], BF16, kind="ExternalInput")
    d_wk = nc.dram_tensor("wk", [C, C], BF16, kind="ExternalInput")
    d_wv = nc.dram_tensor("wv", [C, C], BF16, kind="ExternalInput")
    d_wgT = nc.dram_tensor("wgT", [C, C], BF16, kind="ExternalInput")
    d_woT = nc.dram_tensor("woT", [H, D, C], BF16, kind="ExternalInput")
    d_gb = nc.dram_tensor("gb", [1, H * D], BF16, kind="ExternalInput")
    d_ob = nc.dram_tensor("ob", [1, C], BF16, kind="ExternalInput")
    d_ones = nc.dram_tensor("ones", [128, 256], BF16, kind="ExternalInput")
    d_out = nc.dram_tensor("out", [NQ, C], F32, kind="ExternalOutput")

    with tile.TileContext(nc) as tc:
        with tc.tile_pool(name="persist", bufs=1) as pers:

            # ---------------- persistent SBUF ----------------
            q_projT = [pers.tile([128, NQ], BF16, name=f"q_projT{i}") for i in range(2)]
            k_projT = [pers.tile([128, NK], BF16, name=f"k_projT{i}") for i in range(2)]
            v_aug = pers.tile([128, 16 * H * 33], BF16, name="v_aug")
            gate_t = pers.tile([D, H * NQ], F32, name="gate_t")
            gate_sb = pers.tile([D, H * NQ], F32, name="gate_sb")
            comb = pers.tile([D, H * NQ], BF16, name="comb")
            woT_sb = [pers.tile([D, C], BF16, name=f"woT{h}") for h in range(H)]
            gb_sb = pers.tile([1, H * D], BF16, name="gb_sb")
            ob_sb = pers.tile([1, C], BF16, name="ob_sb")
            ones_sb = pers.tile([128, 256], BF16, name="ones_sb")
            rs16 = pers.tile([33, H * NQ], BF16, name="rs16")
            rsr = pers.tile([D, H * NQ], F32, name="rsr")
            cg = pers.tile([D, NQ], F32, name="cg")
            out_sb = [pers.tile([128, C], F32, name=f"out_sb{i}") for i in range(2)]

            # expb staging: all 8 heads resident (64 KB/partition). Everything
            # rides ONE HWDGE ring (sync) in exact consumption order — FIFO per
            # ring means stage-1 inputs always land before the bias stream, and
            # a single InstDMACopy still fans out over all 16 SDMA slots, so
            # one ring sustains full HBM bandwidth.
            bias_pool = tc.alloc_tile_pool(name="bias_sb", bufs=8)
            bias_tiles = {}

            with tc.tile_pool(name="stage1_sb", bufs=1) as s1, \
                 tc.tile_pool(name="s1_big_ps", bufs=2, space="PSUM") as bigp, \
                 tc.tile_pool(name="s1_small_ps", bufs=2, space="PSUM") as smallp, \
                 tc.tile_pool(name="s1_gate_ps", bufs=2, space="PSUM") as gatep:
                qT_sb = [s1.tile([128, NQ], BF16, name=f"qT{i}") for i in range(2)]
                kT_sb = [s1.tile([128, NK], BF16, name=f"kT{i}") for i in range(2)]
                vT_sb = [s1.tile([128, NK], BF16, name=f"vT{i}") for i in range(2)]
                wq_sb = [s1.tile([128, C], BF16, name=f"wq{i}") for i in range(2)]
                wk_sb = [s1.tile([128, C], BF16, name=f"wk{i}") for i in range(2)]
                wv_sb = [s1.tile([128, C], BF16, name=f"wv{i}") for i in range(2)]
                wgT_sb = [s1.tile([128, C], BF16, name=f"wgT{i}") for i in range(2)]
                for i in range(2):
                    sl = slice(i * 128, (i + 1) * 128)
                    nc.sync.dma_start(qT_sb[i][:], d_qT.ap()[sl, :])
                    nc.sync.dma_start(wq_sb[i][:], d_wq.ap()[sl, :])
                    nc.sync.dma_start(wk_sb[i][:], d_wk.ap()[sl, :])
                    nc.sync.dma_start(wv_sb[i][:], d_wv.ap()[sl, :])
                    nc.sync.dma_start(wgT_sb[i][:], d_wgT.ap()[sl, :])
                nc.sync.dma_start(gb_sb[:], d_gb.ap()[:])
                nc.sync.dma_start(ob_sb[:], d_ob.ap()[:])
                nc.sync.dma_start(ones_sb[:], d_ones.ap()[:])
                for h in range(H):
                    nc.sync.dma_start(woT_sb[h][:], d_woT.ap()[h])
                for i in range(2):
                    sl = slice(i * 128, (i + 1) * 128)
                    nc.sync.dma_start(kT_sb[i][:], d_kT.ap()[sl, :])
                for i in range(2):
                    sl = slice(i * 128, (i + 1) * 128)
                    nc.sync.dma_start(vT_sb[i][:], d_vT.ap()[sl, :])
                for h in range(H):
                    t = bias_pool.tile([128, 16 * NQ], BF16, tag="bias", name="bias_t")
                    nc.sync.dma_start(t[:], d_expb.ap()[h])
                    bias_tiles[h] = t

                # ones columns of v_aug (position 32 of each 33-wide head block)
                v_aug4 = v_aug.rearrange("p (n h e) -> p n h e", n=16, h=H)
                nc.vector.tensor_copy(
                    v_aug4[:, :, :, D:D + 1],
                    ones_sb[:, 0:16 * H].rearrange("p (n h) -> p n h", n=16).unsqueeze(-1))

                # q_projT[mh] [128, NQ] = (Wq_s.T @ q_data.T) chunk (4 heads per tile)
                for mh in range(2):
                    pq = smallp.tile([128, NQ], F32, tag="proj", name="pq")
                    for kc in range(2):
                        nc.tensor.matmul(pq[:], wq_sb[kc][:, mh * 128:(mh + 1) * 128],
                                         qT_sb[kc][:], start=(kc == 0), stop=(kc == 1))
                    nc.vector.tensor_copy(q_projT[mh][:], pq[:])

                # k_projT[mh] [128, NK], evicted by ACT (idle in stage 1)
                for mh in range(2):
                    for half in range(2):
                        pk = bigp.tile([128, 1024], F32, tag="big", name="pk")
                        for nn in range(2):
                            for kc in range(2):
                                nc.tensor.matmul(
                                    pk[:, nn * 512:(nn + 1) * 512],
                                    wk_sb[kc][:, mh * 128:(mh + 1) * 128],
                                    kT_sb[kc][:, half * 1024 + nn * 512:half * 1024 + (nn + 1) * 512],
                                    start=(kc == 0), stop=(kc == 1))
                        nc.scalar.copy(k_projT[mh][:, half * 1024:(half + 1) * 1024], pk[:])

                # v_proj natural layout -> scatter into v_aug (ACT strided copy)
                for g in range(4):
                    pv = bigp.tile([128, 1024], F32, tag="big", name="pv")
                    for nn in range(4):
                        for kc in range(2):
                            nc.tensor.matmul(
                                pv[:, nn * 256:(nn + 1) * 256],
                                vT_sb[kc][:, (g * 4 + nn) * 128:(g * 4 + nn + 1) * 128],
                                wv_sb[kc][:], start=(kc == 0), stop=(kc == 1))
                    nc.scalar.copy(
                        v_aug4[:, g * 4:(g + 1) * 4, :, 0:D],
                        pv[:].rearrange("p (n h d) -> p n h d", n=4, h=H))

                # pin the ACT table set to exp_and_others (holds exp AND tanh)
                # before the first tanh, via a tiny dummy exp
                nc.scalar.activation(gate_t[0:1, 0:1], ones_sb[0:1, 0:1], AF.Exp)

                # gate: per head psum [32, NQ] = WgT_h.T @ qT + gb_h x ones,
                # then tanh(0.5*x) on ACT; sigmoid = 0.5 + 0.5*tanh below
                for h in range(H):
                    pg = gatep.tile([D, NQ], F32, tag="gate", name="pg")
                    for kc in range(2):
                        nc.tensor.matmul(pg[:], wgT_sb[kc][:, h * D:(h + 1) * D],
                                         qT_sb[kc][:], start=(kc == 0), stop=False)
                    nc.tensor.matmul(pg[:], gb_sb[:, h * D:(h + 1) * D],
                                     ones_sb[0:1, 0:NQ], start=False, stop=True)
                    nc.scalar.activation(gate_t[:, h * NQ:(h + 1) * NQ],
                                         pg[:], AF.Tanh, scale=0.5)
                nc.vector.tensor_scalar(gate_sb[:], gate_t[:], 0.5, 0.5,
                                        ALU.mult, ALU.add)

            # ---------------- stage 2+3: attention ----------------
            wTe_pool = tc.alloc_tile_pool(name="wTe_sb", bufs=2)
            wT_pool = tc.alloc_tile_pool(name="wT_sb", bufs=2)
            pl_pool = tc.alloc_tile_pool(name="pl", bufs=3, space="PSUM")
            pc_pool = tc.alloc_tile_pool(name="pc", bufs=2, space="PSUM")

            for th in range(2):
                for i in range(4):
                    h = 4 * th + i
                    po = i * 32
                    wTe = wTe_pool.tile([128, 16 * NQ], BF16, tag="wTe", name="wTe")
                    wT = wT_pool.tile([128, 16 * NQ], BF16, tag="wT", name="wT")
                    expb_t = bias_tiles[h]
                    pctx = pc_pool.tile([128, NQ], F32, tag="pc", name="pctx")
                    for qt in range(4):
                        pl = pl_pool.tile([128, 1024], F32, tag="pl", name="pl")
                        for j in range(4):
                            kc = qt * 4 + j
                            nc.tensor.matmul(
                                pl[:, j * NQ:(j + 1) * NQ],
                                k_projT[th][po:po + 32, kc * 128:(kc + 1) * 128],
                                q_projT[th][po:po + 32, :],
                                start=True, stop=True, tile_position=(po, 0))
                        qsl = slice(qt * 1024, (qt + 1) * 1024)
                        nc.scalar.activation(wTe[:, qsl], pl[:], AF.Exp)
                        nc.vector.tensor_mul(wT[:, qsl], wTe[:, qsl], expb_t[:, qsl])
                        for j in range(4):
                            kc = qt * 4 + j
                            nc.tensor.matmul(
                                pctx[0:33, :],
                                v_aug[:, kc * (H * 33) + h * 33: kc * (H * 33) + h * 33 + 33],
                                wT[:, kc * NQ:(kc + 1) * NQ],
                                start=(kc == 0), stop=(kc == 15))
                    # denominators: raw sums sit in row 32 of pctx. Broadcast to
                    # 32 partitions via ones outer product, then fast reciprocal
                    # at partition offset 0.
                    nc.vector.tensor_copy(rs16[32:33, h * NQ:(h + 1) * NQ], pctx[32:33, :])
                    prsb = pc_pool.tile([128, NQ], F32, tag="pc", name="prsb")
                    nc.tensor.matmul(prsb[0:32, :], ones_sb[32:33, 0:32],
                                     rs16[32:33, h * NQ:(h + 1) * NQ],
                                     start=True, stop=True, tile_position=(32, 0))
                    nc.vector.reciprocal_approx_fast(
                        out=rsr[:, h * NQ:(h + 1) * NQ], in_=prsb[0:32, :])
                    # comb = ctx * gate * recip
                    nc.vector.tensor_mul(cg[:], pctx[0:32, :], gate_sb[:, h * NQ:(h + 1) * NQ])
                    nc.vector.tensor_mul(comb[:, h * NQ:(h + 1) * NQ], cg[:],
                                         rsr[:, h * NQ:(h + 1) * NQ])

            # ---------------- stage 4: output projection ----------------
            for qm in range(2):
                pout = pc_pool.tile([128, C], F32, tag="pc", name="pout")
                for h in range(H):
                    nc.tensor.matmul(pout[:],
                                     comb[:, h * NQ + qm * 128: h * NQ + qm * 128 + 128],
                                     woT_sb[h][:], start=(h == 0), stop=False)
                # o_bias outer product: ones[1,128].T @ ob[1,256]
                nc.tensor.matmul(pout[:], ones_sb[0:1, 0:128], ob_sb[:],
                                 start=False, stop=True)
                nc.vector.tensor_copy(out_sb[qm][:], pout[:])
                nc.sync.dma_start(d_out.ap()[qm * 128:(qm + 1) * 128, :], out_sb[qm][:])

            pc_pool.release()
            pl_pool.release()
            wT_pool.release()
            wTe_pool.release()
            bias_pool.release()

    nc.compile()
    return nc


def _prep_in_maps(inputs):
    import ml_dtypes
    BF = ml_dtypes.bfloat16

    q_data = np.asarray(inputs["q_data"], dtype=np.float32)
    k_data = np.asarray(inputs["k_data"], dtype=np.float32)
    v_data = np.asarray(inputs["v_data"], dtype=np.float32)
    pair_bias = np.asarray(inputs["pair_bias"], dtype=np.float32)
    Wq = np.asarray(inputs["Wq"], dtype=np.float32)
    Wk = np.asarray(inputs["Wk"], dtype=np.float32)
    Wv = np.asarray(inputs["Wv"], dtype=np.float32)
    Wg = np.asarray(inputs["Wg"], dtype=np.float32)
    Wo = np.asarray(inputs["Wo"], dtype=np.float32)
    gating_b = np.asarray(inputs["gating_b"], dtype=np.float32)
    o_bias = np.asarray(inputs["o_bias"], dtype=np.float32)

    kT = np.ascontiguousarray(k_data.T).astype(BF)
    vT = np.ascontiguousarray(v_data.T).astype(BF)
    wq = np.ascontiguousarray(Wq * np.float32(SCALE)).astype(BF)
    wk = Wk.astype(BF)
    wv = Wv.astype(BF)
    wgT = np.ascontiguousarray(Wg.T).astype(BF)
    woT = np.ascontiguousarray(Wo.T.reshape(H, D, C)).astype(BF)
    gb = gating_b.reshape(1, H * D).astype(BF)
    ob = o_bias.reshape(1, C).astype(BF)
    ones = np.ones((128, 256), dtype=BF)

    # exp(pair_bias) transposed and tiled to the exact SBUF layout:
    # expb[c][h, p, kc*NQ + q] = exp(pair_bias[h, c*NQ + q, kc*128 + p])
    expb_all = np.exp(pair_bias)  # [H, NQT, NK]

    in_maps = []
    for c in range(8):
        qs = slice(c * NQ, (c + 1) * NQ)
        eb = expb_all[:, qs, :]                      # [H, NQ, NK]
        eb = eb.transpose(0, 2, 1)                   # [H, NK, NQ]
        eb = eb.reshape(H, 16, 128, NQ).transpose(0, 2, 1, 3)  # [H, 128, 16, NQ]
        eb = np.ascontiguousarray(eb.reshape(H, 128, 16 * NQ)).astype(BF)
        in_maps.append(dict(
            qT=np.ascontiguousarray(q_data[qs, :].T).astype(BF),
            kT=kT, vT=vT,
            expb=eb,
            wq=wq, wk=wk, wv=wv, wgT=wgT, woT=woT,
            gb=gb, ob=ob, ones=ones,
        ))
    return in_maps


def _get_nc():
    if "nc" not in _CACHE:
        _CACHE["nc"] = _build_nc()
    return _CACHE["nc"]


def _run(inputs, trace=False, trace_cores=None):
    from concourse import bass_utils
    nc = _get_nc()
    in_maps = _prep_in_maps(inputs)
    kwargs = {}
    if trace:
        kwargs = dict(trace=True, trace_cores=trace_cores or [0])
    res = bass_utils.run_bass_kernel_spmd(nc, in_maps, core_ids=list(range(8)), **kwargs)
    out = np.concatenate([res.results[c]["out"] for c in range(8)], axis=0)
    return out, res


def kernel(**inputs) -> np.ndarray:
    out, _ = _run(inputs)
    return out
